# revision 4
# baseline (speedup 1.0000x reference)
"""Trainium2 Bass kernel for nn_Group_SA_Linear (grouped SA + cross-SA linear
attention transformer). Data-parallel over batch: core b handles feat[b].
Single AllReduce for the cross-block y-mean. All matmuls bf16 -> f32 PSUM.

Self-contained: hardcodes B=8, C=512, N=4096, GP=4.
"""
import numpy as np
import ml_dtypes

import concourse.bass as bass
import concourse.tile as tile
import concourse.mybir as mybir
from concourse import bacc
from concourse.bass_utils import run_bass_kernel_spmd

P = 128
C = 512
N = 4096
NG = 1024
GP = 4
F = 2048
KC = C // P       # 4
NJ = NG // P      # 8
FC = F // P       # 16
NCORES = 8
F32 = mybir.dt.float32
BF16 = mybir.dt.bfloat16
AL = mybir.AluOpType
AF = mybir.ActivationFunctionType
RS = float(1.0 / np.sqrt(C))

_BUILT = {}


def _emit(nc, tc, T):
    """Emit the whole per-core program. T: dict name->dram handle."""
    import contextlib
    ctx = contextlib.ExitStack()
    wp = ctx.enter_context(tc.tile_pool(name="wp", bufs=1))
    work = ctx.enter_context(tc.tile_pool(name="work", bufs=1))
    small = ctx.enter_context(tc.tile_pool(name="small", bufs=1))
    ps = ctx.enter_context(tc.tile_pool(name="ps", bufs=2, space="PSUM"))
    dram = ctx.enter_context(tc.tile_pool(name="dram", bufs=2, space="DRAM"))

    def ldw(name, cols, nchunk):
        t = wp.tile([P, nchunk, cols], BF16, name=name, tag=name)
        nc.sync.dma_start(t[:], T[name][:].rearrange("(k p) m -> p k m", p=P))
        return t

    # --- resident weights ---
    WQK = ldw("twqkt", C, KC)
    WV = ldw("twvt", C, KC)
    WPH = ldw("twphit", C, KC)
    CWQ = ldw("cwqt", C, KC)
    CWK = ldw("cwkt", C, KC)
    CWV = ldw("cwvt", C, KC)
    CWPH = ldw("cwphit", C, KC)

    def ldvec(name, nchunk):
        t = wp.tile([P, nchunk], F32, name=name, tag=name)
        nc.sync.dma_start(t[:], T[name][:])
        return t

    VEC = {k: ldvec(k, FC if k in ("tf1b", "cf1b") else KC)
           for k in ("tg1", "tb1", "tf1b", "tf2b", "tg2", "tb2",
                     "cg1", "cb1", "cf1b", "cf2b", "cg2", "cb2")}

    ones = wp.tile([P, 1], BF16, name="ones", tag="ones")
    nc.vector.memset(ones[:], 1.0)

    outr = T["out"][:].rearrange("(kc p) (j t g) -> p kc j t g", p=P, t=256, g=GP)

    # ---------- helpers ----------
    def proj_normal(dst, wt, rhs_fn, act, nblk, bw):
        """dst[:,mc,b*bw:+bw] = act( sum_kc wt[:,kc,mc*P:+P].T @ rhs_fn(kc,b) )"""
        for mc in range(KC):
            for b in range(nblk):
                pt = ps.tile([P, 512], F32, name="mm", tag="mm")[:, :bw]
                for kc in range(KC):
                    nc.tensor.matmul(pt, wt[:, kc, mc * P:(mc + 1) * P],
                                     rhs_fn(kc, b), start=(kc == 0), stop=(kc == KC - 1))
                d = dst[:, mc, b * bw:(b + 1) * bw]
                if act == "phi":
                    nc.vector.tensor_scalar(d, pt, 0.0, 1.0, AL.max, AL.add)
                else:
                    nc.scalar.copy(d, pt)

    def proj_T(dst, wt, lhs_fn, act):
        """dst[:,j,:] = act( lhs_fn(kc,j).T @ wt[:,kc,:] summed over kc )"""
        for j in range(NJ):
            pt = ps.tile([P, 512], F32, name="mm", tag="mm")
            for kc in range(KC):
                nc.tensor.matmul(pt, lhs_fn(kc, j), wt[:, kc, :],
                                 start=(kc == 0), stop=(kc == KC - 1))
            d = dst[:, j, :]
            if act == "phi":
                nc.vector.tensor_scalar(d, pt, 0.0, 1.0, AL.max, AL.add)
            else:
                nc.scalar.copy(d, pt)

    def row_stat_mm(dst_row, src, scale):
        """dst_row [1,NG] f32 = scale * column-sums of src [P,KC,NG] (over all C)."""
        for nh in range(2):
            pt = ps.tile([1, 512], F32, name="st", tag="st")
            for kc in range(KC):
                nc.tensor.matmul(pt, ones[:], src[:, kc, nh * 512:(nh + 1) * 512],
                                 start=(kc == 0), stop=(kc == KC - 1))
            nc.scalar.mul(dst_row[:, nh * 512:(nh + 1) * 512], pt, scale)

    def bcast_half(row, nh, name):
        """row [1,NG] f32 -> [P,512] f32 broadcast of its nh-th half (DRAM trip)."""
        d = dram.tile([1, NG], F32, name="d_" + name, tag="drow")
        nc.sync.dma_start(d[:], row[:])
        t = work.tile([P, 512], F32, name=name, tag="bc", bufs=3)
        nc.sync.dma_start(t[:], d[:, nh * 512:(nh + 1) * 512].to_broadcast((P, 512)))
        return t

    def bcast_full(row, name):
        d = dram.tile([1, NG], F32, name="d_" + name, tag="drow")
        nc.sync.dma_start(d[:], row[:])
        t = work.tile([P, NG], F32, name=name, tag="bcf", bufs=2)
        nc.sync.dma_start(t[:], d[:].to_broadcast((P, NG)))
        return t

    def softmax_alpha(src_norm, tagpfx):
        """alpha [1,NG] f32 (=softmax(qg . src)*NG) and alphaT [P,NJ,1] f32."""
        qg = small.tile([P, KC, 1], F32, name=tagpfx + "qg", tag="qg")
        for kc in range(KC):
            nc.vector.tensor_reduce(qg[:, kc, :], src_norm[:, kc, :],
                                    axis=mybir.AxisListType.X, op=AL.add)
        qgb = small.tile([P, KC, 1], BF16, name=tagpfx + "qgb", tag="qgb")
        nc.scalar.mul(qgb[:], qg[:], 1.0 / NG)
        s = small.tile([1, NG], F32, name=tagpfx + "s", tag="rowa")
        for nh in range(2):
            pt = ps.tile([1, 512], F32, name="st", tag="st")
            for kc in range(KC):
                nc.tensor.matmul(pt, qgb[:, kc, :], src_norm[:, kc, nh * 512:(nh + 1) * 512],
                                 start=(kc == 0), stop=(kc == KC - 1))
            nc.scalar.copy(s[:, nh * 512:(nh + 1) * 512], pt)
        mx = small.tile([1, 1], F32, name=tagpfx + "mx", tag="mx")
        nc.vector.tensor_reduce(mx[:], s[:], axis=mybir.AxisListType.X, op=AL.max)
        nmx = small.tile([1, 1], F32, name=tagpfx + "nmx", tag="nmx")
        nc.scalar.mul(nmx[:], mx[:], -1.0)
        nc.scalar.activation(s[:], s[:], AF.Exp, bias=nmx[:], scale=1.0)
        se = small.tile([1, 1], F32, name=tagpfx + "se", tag="se")
        nc.vector.tensor_reduce(se[:], s[:], axis=mybir.AxisListType.X, op=AL.add)
        rn = small.tile([1, 1], F32, name=tagpfx + "rn", tag="rn")
        nc.vector.reciprocal(rn[:], se[:])
        nc.scalar.mul(rn[:], rn[:], float(NG))
        nc.vector.tensor_scalar_mul(s[:], s[:], rn[:])
        # alphaT via DRAM roundtrip
        d = dram.tile([1, NG], F32, name=tagpfx + "da", tag="drow")
        nc.sync.dma_start(d[:], s[:])
        aT = small.tile([P, NJ, 1], F32, name=tagpfx + "aT", tag="aT")
        nc.sync.dma_start(aT[:, :, 0], d[0, :].rearrange("(j p) -> p j", p=P))
        return s, aT

    def kv_ksum(kT, vT, tagpfx):
        kv = work.tile([P, KC, C], BF16, name=tagpfx + "kv", tag="kv")
        for cc in range(KC):
            pt = ps.tile([P, 512], F32, name="mm", tag="mm")
            for j in range(NJ):
                nc.tensor.matmul(pt, kT[:, j, cc * P:(cc + 1) * P], vT[:, j, :],
                                 start=(j == 0), stop=(j == NJ - 1))
            nc.scalar.mul(kv[:, cc, :], pt, RS)
        ksb = small.tile([P, KC, 1], BF16, name=tagpfx + "ksb", tag="ksb")
        for cc in range(KC):
            pk = ps.tile([P, 1], F32, name="ks", tag="ks")
            for j in range(NJ):
                nc.tensor.matmul(pk, kT[:, j, cc * P:(cc + 1) * P], ones[:],
                                 start=(j == 0), stop=(j == NJ - 1))
            nc.scalar.copy(ksb[:, cc, :], pk)
        return kv, ksb

    def z_row(qn, ksb, tagpfx):
        s2 = small.tile([1, NG], F32, name=tagpfx + "s2", tag="rowz")
        for nh in range(2):
            pt = ps.tile([1, 512], F32, name="st", tag="st")
            for kc in range(KC):
                nc.tensor.matmul(pt, ksb[:, kc, :], qn[:, kc, nh * 512:(nh + 1) * 512],
                                 start=(kc == 0), stop=(kc == KC - 1))
            nc.scalar.copy(s2[:, nh * 512:(nh + 1) * 512], pt)
        nc.vector.tensor_scalar_add(s2[:], s2[:], 1e-6)
        nc.vector.reciprocal(s2[:], s2[:])
        return s2

    def ln_stats(xb, xs, tagpfx):
        mu = small.tile([1, NG], F32, name=tagpfx + "mu", tag="rowa")
        ms = small.tile([1, NG], F32, name=tagpfx + "ms", tag="rms")
        row_stat_mm(mu, xb, 1.0 / C)
        row_stat_mm(ms, xs, 1.0 / C)
        mu2 = small.tile([1, NG], F32, name=tagpfx + "mu2", tag="rowz")
        nc.vector.tensor_mul(mu2[:], mu[:], mu[:])
        nc.vector.tensor_tensor(ms[:], ms[:], mu2[:], AL.subtract)
        nc.vector.tensor_scalar_add(ms[:], ms[:], 1e-6)
        nc.scalar.sqrt(ms[:], ms[:])
        nc.vector.reciprocal(ms[:], ms[:])
        return mu, ms  # mean row, rstd row

    def ffn_ln(x2, x2s, g1, b1, f1t, f1b, f2t, f2b, g2, b2, out_fn, dst_bf, tp):
        mu, rstd = ln_stats(x2, x2s, tp + "l1")
        h = work.tile([P, KC, NG], BF16, name=tp + "h", tag="tB")
        for nh in range(2):
            mub = bcast_half(mu, nh, tp + "mub%d" % nh)
            rsb = bcast_half(rstd, nh, tp + "rsb%d" % nh)
            sl = slice(nh * 512, nh * 512 + 512)
            for kc in range(KC):
                t1 = work.tile([P, 512], F32, name="t1", tag="t1", bufs=2)
                nc.vector.tensor_tensor(t1[:], x2[:, kc, sl], mub[:], AL.subtract)
                t2 = work.tile([P, 512], F32, name="t2", tag="t2", bufs=2)
                nc.vector.tensor_mul(t2[:], t1[:], rsb[:])
                nc.vector.tensor_scalar(h[:, kc, sl], t2[:], g1[:, kc:kc + 1],
                                        b1[:, kc:kc + 1], AL.mult, AL.add)
        h3 = work.tile([P, KC, NG], BF16, name=tp + "h3", tag="tD")
        h3s = work.tile([P, KC, NG], BF16, name=tp + "h3s", tag="tC")
        for qt in range(4):  # quarter blocks of n (256 cols)
            sl = slice(qt * 256, qt * 256 + 256)
            h1 = work.tile([P, FC, 256], BF16, name="h1", tag="tE", bufs=1)
            for fc in range(FC):
                pt = ps.tile([P, 512], F32, name="mm", tag="mm")[:, :256]
                for kc in range(KC):
                    nc.tensor.matmul(pt, f1t[:, kc, fc * P:(fc + 1) * P],
                                     h[:, kc, sl], start=(kc == 0), stop=(kc == KC - 1))
                nc.scalar.activation(h1[:, fc, :], pt, AF.Relu,
                                     bias=f1b[:, fc:fc + 1], scale=1.0)
            for cc in range(KC):
                pt = ps.tile([P, 512], F32, name="mm", tag="mm")[:, :256]
                for fc in range(FC):
                    nc.tensor.matmul(pt, f2t[:, fc, cc * P:(cc + 1) * P],
                                     h1[:, fc, :], start=(fc == 0), stop=(fc == FC - 1))
                nc.vector.scalar_tensor_tensor(h3[:, cc, sl], pt, f2b[:, cc:cc + 1],
                                               h[:, cc, sl], AL.add, AL.add)
                nc.vector.tensor_mul(h3s[:, cc, sl], h3[:, cc, sl], h3[:, cc, sl])
        mu2r, rstd2 = ln_stats(h3, h3s, tp + "l2")
        for nh in range(2):
            mub = bcast_half(mu2r, nh, tp + "mu2b%d" % nh)
            rsb = bcast_half(rstd2, nh, tp + "rs2b%d" % nh)
            sl = slice(nh * 512, nh * 512 + 512)
            for kc in range(KC):
                t1 = work.tile([P, 512], F32, name="t1", tag="t1", bufs=2)
                nc.vector.tensor_tensor(t1[:], h3[:, kc, sl], mub[:], AL.subtract)
                t2 = work.tile([P, 512], F32, name="t2", tag="t2", bufs=2)
                nc.vector.tensor_mul(t2[:], t1[:], rsb[:])
                fo = work.tile([P, 512], F32, name="fo", tag="fo", bufs=2)
                nc.scalar.activation(fo[:], t2[:], AF.Relu,
                                     scale=g2[:, kc:kc + 1], bias=b2[:, kc:kc + 1])
                if dst_bf is not None:
                    nc.vector.tensor_copy(dst_bf[:, kc, sl], fo[:])
                out_fn(kc, nh, fo)

    # ---------- SA FFN weights (resident across 4 groups) ----------
    f1t_sa = wp.tile([P, KC, F], BF16, name="f1t_sa", tag="f1t_sa")
    nc.sync.dma_start(f1t_sa[:], T["tf1wt"][:].rearrange("(k p) m -> p k m", p=P))
    f2t_sa = wp.tile([P, FC, C], BF16, name="f2t_sa", tag="f2t_sa")
    nc.sync.dma_start(f2t_sa[:], T["tf2wt"][:].rearrange("(k p) m -> p k m", p=P))

    fbf = []
    # ---------- SA block: 4 groups ----------
    for g in range(GP):
        xt = work.tile([P, KC, NG], BF16, name="xt%d" % g, tag="xt", bufs=1)
        nc.sync.dma_start(xt[:], T["xg"][g].rearrange("(kc p) n -> p kc n", p=P))

        q = work.tile([P, KC, NG], BF16, name="q%d" % g, tag="tD")
        proj_normal(q, WQK, lambda kc, b: xt[:, kc, b * 512:(b + 1) * 512], "phi", 2, 512)
        qT = work.tile([P, NJ, C], BF16, name="qT%d" % g, tag="tA")
        proj_T(qT, WQK, lambda kc, j: xt[:, kc, j * P:(j + 1) * P], "phi")
        vT = work.tile([P, NJ, C], BF16, name="vT%d" % g, tag="tB")
        proj_T(vT, WV, lambda kc, j: xt[:, kc, j * P:(j + 1) * P], None)
        px = work.tile([P, KC, NG], BF16, name="px%d" % g, tag="tF")
        proj_normal(px, WPH, lambda kc, b: xt[:, kc, b * 512:(b + 1) * 512], None, 2, 512)

        alpha, aT = softmax_alpha(q, "sa%d" % g)
        kT = work.tile([P, NJ, C], BF16, name="kT%d" % g, tag="tC")
        for j in range(NJ):
            nc.vector.tensor_scalar_mul(kT[:, j, :], qT[:, j, :], aT[:, j, :])
        kv, ksb = kv_ksum(kT, vT, "sa%d" % g)
        zr = z_row(q, ksb, "sa%d" % g)

        x2 = work.tile([P, KC, NG], BF16, name="x2_%d" % g, tag="tA")
        x2s = work.tile([P, KC, NG], BF16, name="x2s%d" % g, tag="tC")
        for nh in range(2):
            zb = bcast_half(zr, nh, "zb%d_%d" % (g, nh))
            sl = slice(nh * 512, nh * 512 + 512)
            for dc in range(KC):
                pt = ps.tile([P, 512], F32, name="mm", tag="mm")
                for kc in range(KC):
                    nc.tensor.matmul(pt, kv[:, kc, dc * P:(dc + 1) * P],
                                     q[:, kc, sl], start=(kc == 0), stop=(kc == KC - 1))
                t1 = work.tile([P, 512], F32, name="t1", tag="t1", bufs=2)
                nc.vector.tensor_mul(t1[:], pt, zb[:])
                t2 = work.tile([P, 512], F32, name="t2", tag="t2", bufs=2)
                nc.vector.tensor_mul(t2[:], t1[:], px[:, dc, sl])
                nc.vector.tensor_tensor(x2[:, dc, sl], t2[:], xt[:, dc, sl], AL.add)
                nc.vector.tensor_mul(x2s[:, dc, sl], x2[:, dc, sl], x2[:, dc, sl])

        fb = wp.tile([P, KC, NG], BF16, name="fbf%d" % g, tag="fbf%d" % g)
        fbf.append(fb)

        def sa_out(kc, nh, fo, g=g):
            # passthrough output for contiguous groups j=1..3 (j=0 kept in fbf)
            for sub in range(2):
                j = nh * 2 + sub
                if j >= 1:
                    nc.sync.dma_start(outr[:, kc, j, :, g],
                                      fo[:, sub * 256:(sub + 1) * 256])

        ffn_ln(x2, x2s, VEC["tg1"], VEC["tb1"], f1t_sa, VEC["tf1b"], f2t_sa,
               VEC["tf2b"], VEC["tg2"], VEC["tb2"], sa_out, fb, "g%d" % g)

    # ---------- Cross block (G-space) ----------
    k0 = work.tile([P, KC, NG], BF16, name="k0", tag="tD")
    proj_normal(k0, CWK, lambda kc, b: fbf[b][:, kc, 0:256], "phi", 4, 256)
    k0T = work.tile([P, NJ, C], BF16, name="k0T", tag="tA")
    proj_T(k0T, CWK, lambda kc, j: fbf[j // 2][:, kc, (j % 2) * P:(j % 2) * P + P], "phi")
    v0T = work.tile([P, NJ, C], BF16, name="v0T", tag="tB")
    proj_T(v0T, CWV, lambda kc, j: fbf[j // 2][:, kc, (j % 2) * P:(j % 2) * P + P], None)

    alpha, aT = softmax_alpha(k0, "cx")
    kT = work.tile([P, NJ, C], BF16, name="kTc", tag="tC")
    for j in range(NJ):
        nc.vector.tensor_scalar_mul(kT[:, j, :], k0T[:, j, :], aT[:, j, :])
    kv, ksb = kv_ksum(kT, v0T, "cx")

    px0 = work.tile([P, KC, NG], BF16, name="px0", tag="px0")
    proj_normal(px0, CWPH, lambda kc, b: fbf[b][:, kc, 0:256], None, 4, 256)

    yacc = work.tile([P, KC, NG], F32, name="yacc", tag="yacc")
    for j in (1, 2, 3):
        qj = work.tile([P, KC, NG], BF16, name="qj%d" % j, tag="tD")
        proj_normal(qj, CWQ,
                    lambda kc, b: fbf[b][:, kc, j * 256:(j + 1) * 256], "phi", 4, 256)
        pxj = work.tile([P, KC, NG], BF16, name="pxj%d" % j, tag="tF")
        proj_normal(pxj, CWPH,
                    lambda kc, b: fbf[b][:, kc, j * 256:(j + 1) * 256], None, 4, 256)
        zr = z_row(qj, ksb, "cx%d" % j)
        for nh in range(2):
            zb = bcast_half(zr, nh, "zbc%d_%d" % (j, nh))
            sl = slice(nh * 512, nh * 512 + 512)
            for dc in range(KC):
                pt = ps.tile([P, 512], F32, name="mm", tag="mm")
                for kc in range(KC):
                    nc.tensor.matmul(pt, kv[:, kc, dc * P:(dc + 1) * P],
                                     qj[:, kc, sl], start=(kc == 0), stop=(kc == KC - 1))
                t1 = work.tile([P, 512], F32, name="t1", tag="t1", bufs=2)
                nc.vector.tensor_mul(t1[:], pt, zb[:])
                if j == 1:
                    nc.vector.tensor_mul(yacc[:, dc, sl], t1[:], pxj[:, dc, sl])
                else:
                    t2 = work.tile([P, 512], F32, name="t2", tag="t2", bufs=2)
                    nc.vector.tensor_mul(t2[:], t1[:], pxj[:, dc, sl])
                    nc.vector.tensor_tensor(yacc[:, dc, sl], yacc[:, dc, sl], t2[:], AL.add)

    # ---------- AllReduce of yacc ----------
    cin = dram.tile([C, NG], F32, name="cc_in", tag="cc_in")
    cout = dram.tile([C, NG], F32, name="cc_out", tag="cc_out")
    nc.sync.dma_start(cin[:].rearrange("(kc p) n -> p kc n", p=P), yacc[:])
    nc.gpsimd.collective_compute(
        "AllReduce", AL.add, replica_groups=[list(range(NCORES))],
        ins=[cin.opt()], outs=[cout.opt()])
    ym = work.tile([P, KC, NG], F32, name="ym", tag="yacc")
    nc.sync.dma_start(ym[:], cout[:].rearrange("(kc p) n -> p kc n", p=P))

    # cross FFN weights (reuse SA slots is not safe -> own tags)
    f1t_cx = wp.tile([P, KC, F], BF16, name="f1t_cx", tag="f1t_sa")
    nc.sync.dma_start(f1t_cx[:], T["cf1wt"][:].rearrange("(k p) m -> p k m", p=P))
    f2t_cx = wp.tile([P, FC, C], BF16, name="f2t_cx", tag="f2t_sa")
    nc.sync.dma_start(f2t_cx[:], T["cf2wt"][:].rearrange("(k p) m -> p k m", p=P))

    # x2c = G0 + ym/24 * px0   (G0 block g = fbf[g][:, :, 0:256])
    x2c = work.tile([P, KC, NG], BF16, name="x2c", tag="tA")
    x2cs = work.tile([P, KC, NG], BF16, name="x2cs", tag="tC")
    for kc in range(KC):
        for g in range(GP):
            sl = slice(g * 256, g * 256 + 256)
            t1 = work.tile([P, 512], F32, name="t1", tag="t1", bufs=2)[:, :256]
            nc.scalar.mul(t1, ym[:, kc, sl], 1.0 / 24.0)
            t2 = work.tile([P, 512], F32, name="t2", tag="t2", bufs=2)[:, :256]
            nc.vector.tensor_mul(t2, t1, px0[:, kc, sl])
            nc.vector.tensor_tensor(x2c[:, kc, sl], t2, fbf[g][:, kc, 0:256], AL.add)
            nc.vector.tensor_mul(x2cs[:, kc, sl], x2c[:, kc, sl], x2c[:, kc, sl])

    def cx_out(kc, nh, fo):
        for sub in range(2):
            g = nh * 2 + sub
            nc.sync.dma_start(outr[:, kc, 0, :, g],
                              fo[:, sub * 256:(sub + 1) * 256])

    ffn_ln(x2c, x2cs, VEC["cg1"], VEC["cb1"], f1t_cx, VEC["cf1b"], f2t_cx,
           VEC["cf2b"], VEC["cg2"], VEC["cb2"], cx_out, None, "cx")
    ctx.close()


def _build():
    if "nc" in _BUILT:
        return _BUILT["nc"]
    nc = bacc.Bacc("TRN2", target_bir_lowering=False, debug=False,
                   num_devices=NCORES)
    T = {}
    T["xg"] = nc.declare_dram_parameter("xg", [GP, C, NG], BF16, isOutput=False)
    for nm in ("twqkt", "twvt", "twphit", "cwqt", "cwkt", "cwvt", "cwphit"):
        T[nm] = nc.declare_dram_parameter(nm, [C, C], BF16, isOutput=False)
    T["tf1wt"] = nc.declare_dram_parameter("tf1wt", [C, F], BF16, isOutput=False)
    T["tf2wt"] = nc.declare_dram_parameter("tf2wt", [F, C], BF16, isOutput=False)
    T["cf1wt"] = nc.declare_dram_parameter("cf1wt", [C, F], BF16, isOutput=False)
    T["cf2wt"] = nc.declare_dram_parameter("cf2wt", [F, C], BF16, isOutput=False)
    for nm in ("tg1", "tb1", "tf2b", "tg2", "tb2", "cg1", "cb1", "cf2b", "cg2", "cb2"):
        T[nm] = nc.declare_dram_parameter(nm, [P, KC], F32, isOutput=False)
    for nm in ("tf1b", "cf1b"):
        T[nm] = nc.declare_dram_parameter(nm, [P, FC], F32, isOutput=False)
    T["out"] = nc.declare_dram_parameter("out", [C, N], F32, isOutput=True)
    with tile.TileContext(nc) as tc:
        _emit(nc, tc, T)
    nc.finalize()
    _BUILT["nc"] = nc
    return nc


def _prep_shared(inputs):
    BF = ml_dtypes.bfloat16

    def wt(a):  # [out,in] -> transposed bf16
        return np.ascontiguousarray(np.asarray(a, np.float32).T).astype(BF)

    def vec(a, nch):  # [len] -> [P, nch] f32
        return np.ascontiguousarray(np.asarray(a, np.float32).reshape(nch, P).T)

    sh = {
        "twqkt": wt(inputs["tw_qk"]), "twvt": wt(inputs["tw_v"]),
        "twphit": wt(inputs["tw_phi"]),
        "cwqt": wt(inputs["cw_q"]), "cwkt": wt(inputs["cw_k"]),
        "cwvt": wt(inputs["cw_v"]), "cwphit": wt(inputs["cw_phi"]),
        "tf1wt": wt(inputs["tf1w"]), "tf2wt": wt(inputs["tf2w"]),
        "cf1wt": wt(inputs["cf1w"]), "cf2wt": wt(inputs["cf2w"]),
        "tg1": vec(inputs["tg1"], KC), "tb1": vec(inputs["tb1"], KC),
        "tf1b": vec(inputs["tf1b"], FC), "tf2b": vec(inputs["tf2b"], KC),
        "tg2": vec(inputs["tg2"], KC), "tb2": vec(inputs["tb2"], KC),
        "cg1": vec(inputs["cg1"], KC), "cb1": vec(inputs["cb1"], KC),
        "cf1b": vec(inputs["cf1b"], FC), "cf2b": vec(inputs["cf2b"], KC),
        "cg2": vec(inputs["cg2"], KC), "cb2": vec(inputs["cb2"], KC),
    }
    return sh


def kernel(**inputs):
    nc = _build()
    sh = _prep_shared(inputs)
    feat = np.asarray(inputs["feat"], np.float32)          # [8, 512, 4096]
    BF = ml_dtypes.bfloat16
    in_maps = []
    for b in range(NCORES):
        # interval grouping: group g takes cols g, g+4, ... -> [GP, C, NG]
        xg = np.ascontiguousarray(
            feat[b].reshape(C, NG, GP).transpose(2, 0, 1)).astype(BF)
        m = dict(sh)
        m["xg"] = xg
        in_maps.append(m)
    res = run_bass_kernel_spmd(nc, in_maps, list(range(NCORES)))
    out = np.stack([np.asarray(res.results[b]["out"], np.float32)
                    for b in range(NCORES)], axis=0)
    return out
